# revision 2
# baseline (speedup 1.0000x reference)
"""Trainium2 Bass kernel v2 for nn_BipartiteRemap (GNN attention message passing).

Key insight vs v1: the attention logit a_e = att.(W x_src + b) depends ONLY on
the source node, so exp(prelu(a)) = e_j and the per-edge message
m_e = e_j * (W x_j + b) are PER-NODE quantities.

  Launch A: per-node table V[j] = [m_j (128 f16) | e_j (f16) | pad] (512B rows).
  Launch B: per 128-edge block: dma_gather of V rows + one-hot(is_equal) +
    ONE accumulating matmul [128 slots x 129 cols] -> PSUM (num | den).
    Eviction: y = num / den.  Launch B is gather-descgen bound (Q7 ~6.5ns/idx);
    all other engines hide under it.
"""

import os
import sys

for _p in ("/opt/trn_rl_repo",):
    if _p not in sys.path:
        sys.path.insert(0, _p)

import numpy as np

import concourse.bass as bass
import concourse.bacc as bacc
import concourse.mybir as mybir
import concourse.tile as tile
from concourse import library_config
from concourse.bass_utils import run_bass_kernel_spmd

F32 = mybir.dt.float32
F16 = mybir.dt.float16
F8 = mybir.dt.float8e4
I16 = mybir.dt.int16
AF = mybir.ActivationFunctionType
OP = mybir.AluOpType

ROW = 256          # fp16 elems per V row (512 B)
VC = 129           # used cols per V row (m 128 + e 1)


class Cfg:
    def __init__(self, n_src=100_000, n_out=100_000, n_edges=1_600_000, c=128,
                 n_cores=8, chunk=128, super_chunks=5, group=32768,
                 cap_sigma=2.0, gpiece_idx=1024, n_queues=4, scratch=16384):
        self.n_queues = n_queues
        self.scratch = scratch
        self.n_src, self.n_out, self.n_edges, self.c = n_src, n_out, n_edges, c
        self.n_cores = n_cores
        self.chunk = chunk
        self.tpc = n_out // n_cores              # targets per core
        self.nch = -(-self.tpc // chunk)         # chunks per core
        self.super = super_chunks
        self.nsup = -(-self.nch // self.super)
        self.group = group
        self.ngrp = -(-n_src // group)
        self.grp_sizes = [min(group, n_src - g * group) for g in range(self.ngrp)]
        epc = n_edges / n_cores / self.nch       # mean edges per chunk
        self.caps = []
        for g in range(self.ngrp):
            mu = epc * self.grp_sizes[g] / n_src
            sig = mu ** 0.5
            cap = int(-(-(mu + cap_sigma * sig) // 128) * 128)
            self.caps.append(max(cap, 128))
        self.bpc = sum(self.caps) // 128         # blocks per chunk
        self.nblk = self.nch * self.bpc          # blocks per core
        self.icols = self.nblk * 128 // 16       # int16 idx columns
        self.gpiece_idx = gpiece_idx             # max idxs per dma_gather
        self.sup_chunks = [list(range(s * self.super, min((s + 1) * self.super, self.nch)))
                           for s in range(self.nsup)]

    def sup_blocks(self, s):
        """(g, ci_in_sup, j) in slab order for superchunk s (g-major)."""
        out = []
        for g in range(self.ngrp):
            for ci in range(len(self.sup_chunks[s])):
                for j in range(self.caps[g] // 128):
                    out.append((g, ci, j))
        return out


CFG = Cfg()


# ---------------------------------------------------------------- launch A ---

def build_nc_A(cfg: Cfg, sup=7):
    nc = bacc.Bacc("TRN2", target_bir_lowering=False, debug=False,
                   enable_asserts=False, num_devices=cfg.n_cores)
    c = cfg.c
    tpc_src = -(-cfg.n_src // cfg.n_cores // 128) * 128   # host pads to 12544
    nb = tpc_src // 128
    assert nb % sup == 0
    x_d = nc.dram_tensor("x16", [c, tpc_src], F16, kind="ExternalInput")
    wt_d = nc.dram_tensor("WT16", [c, c], F16, kind="ExternalInput")
    attb_d = nc.dram_tensor("ATTB", [128, c], F32, kind="ExternalInput")
    brow_d = nc.dram_tensor("BROW", [128, c], F32, kind="ExternalInput")
    cal_d = nc.dram_tensor("CAL", [128, 2], F32, kind="ExternalInput")  # c0 | alpha
    v_d = nc.dram_tensor("V", [tpc_src, ROW], F16, kind="ExternalOutput")
    with tile.TileContext(nc) as tc:
        import contextlib
        with contextlib.ExitStack() as ctx:
            cpool = ctx.enter_context(tc.tile_pool(name="c", bufs=1))
            ppool = ctx.enter_context(tc.tile_pool(name="p", bufs=2, space="PSUM"))
            spool = ctx.enter_context(tc.tile_pool(name="s", bufs=3))
            vpool = ctx.enter_context(tc.tile_pool(name="v", bufs=3))
            wt = cpool.tile([c, c], F16, tag="wt")
            nc.sync.dma_start(wt[:], wt_d[:])
            attb = cpool.tile([128, c], F32, tag="attb")
            nc.sync.dma_start(attb[:], attb_d[:])
            brow = cpool.tile([128, c], F32, tag="brow")
            nc.sync.dma_start(brow[:], brow_d[:])
            cal = cpool.tile([128, 2], F32, tag="cal")
            nc.sync.dma_start(cal[:], cal_d[:])
            c0col = cal[:, 0:1]
            alcol = cal[:, 1:2]
            x_sb = cpool.tile([c, tpc_src], F16, tag="x")
            nc.sync.dma_start(x_sb[:], x_d[:])
            for s in range(nb // sup):
                ps = ppool.tile([128, sup, c], F32, tag="ps")
                for i in range(sup):
                    o = (s * sup + i) * 128
                    nc.tensor.matmul(out=ps[:, i, :],
                                     lhsT=x_sb[:, o:o + 128], rhs=wt[:],
                                     start=True, stop=True)
                prod = spool.tile([128, sup, c], F32, tag="prod")
                nc.vector.tensor_tensor(
                    out=prod[:], in0=ps[:],
                    in1=attb[:].unsqueeze(1).broadcast_to([128, sup, c]),
                    op=OP.mult)
                acol = spool.tile([128, sup], F32, tag="acol")
                nc.vector.tensor_reduce(out=acol[:], in_=prod[:],
                                        axis=mybir.AxisListType.X, op=OP.add)
                lcol = spool.tile([128, sup], F32, tag="lcol")
                nc.vector.tensor_scalar(
                    out=lcol[:], in0=acol[:], scalar1=c0col,
                    scalar2=None, op0=OP.add)
                pcol = spool.tile([128, sup], F32, tag="pcol")
                nc.vector.scalar_tensor_tensor(
                    out=pcol[:], in0=lcol[:], scalar=alcol,
                    in1=lcol[:], op0=OP.mult, op1=OP.max)
                ecol = spool.tile([128, sup], F32, tag="ecol")
                nc.scalar.activation(ecol[:], pcol[:], AF.Exp)
                fix = spool.tile([128, sup, c], F32, tag="fix")
                nc.vector.tensor_tensor(
                    out=fix[:], in0=ps[:],
                    in1=brow[:].unsqueeze(1).broadcast_to([128, sup, c]),
                    op=OP.add)
                vt = vpool.tile([128, sup, ROW], F16, tag="vt")
                nc.vector.tensor_tensor(
                    out=vt[:, :, 0:c], in0=fix[:],
                    in1=ecol[:].unsqueeze(-1).broadcast_to([128, sup, c]),
                    op=OP.mult)
                nc.vector.tensor_copy(vt[:, :, c:c + 1], ecol[:].unsqueeze(-1))
                nc.sync.dma_start(
                    v_d[s * sup * 128:(s + 1) * sup * 128, :].rearrange(
                        "(b p) r -> p b r", p=128),
                    vt[:])
    nc.compile()
    return nc


# ---------------------------------------------------------------- launch B ---

def build_nc_B(cfg: Cfg, _stage="full"):
    nc = bacc.Bacc("TRN2", target_bir_lowering=False, debug=False,
                   enable_asserts=False, num_devices=cfg.n_cores,
                   num_swdge_queues=cfg.n_queues,
                   dynamic_dma_scratch_size=cfg.scratch)
    c = cfg.c
    if _stage != "full":
        dbg_d = nc.dram_tensor("DBG", [128, cfg.nblk, ROW], I16, kind="ExternalOutput")
    v_d = nc.dram_tensor("V", [cfg.n_src, ROW], F16, kind="ExternalInput")
    idx_d = nc.dram_tensor("IDX", [128, cfg.icols], I16, kind="ExternalInput")
    lt_d = nc.dram_tensor("LT", [128, cfg.nblk], F16, kind="ExternalInput")
    iota_d = nc.dram_tensor("IOTA16", [128, 128], F16, kind="ExternalInput")
    y_d = nc.dram_tensor("Y", [cfg.nch * cfg.chunk, c], F32, kind="ExternalOutput")
    den_d = nc.dram_tensor("DEN", [128, cfg.nch], F32, kind="ExternalOutput")

    with tile.TileContext(nc) as tc:
        import contextlib
        with contextlib.ExitStack() as ctx:
            cpool = ctx.enter_context(tc.tile_pool(name="const", bufs=1))
            slabp = ctx.enter_context(tc.tile_pool(name="slab", bufs=2))
            op_ = ctx.enter_context(tc.tile_pool(name="oh", bufs=2))
            psp = ctx.enter_context(tc.tile_pool(name="ps", bufs=1, space="PSUM"))
            evp = ctx.enter_context(tc.tile_pool(name="ev", bufs=4))
            yp = ctx.enter_context(tc.tile_pool(name="y", bufs=3))

            nc.gpsimd.load_library(library_config.mlp)

            idx_sb = cpool.tile([128, cfg.icols], I16, tag="idx")
            nc.sync.dma_start(idx_sb[:], idx_d[:])
            lt = cpool.tile([128, cfg.nblk], F16, tag="lt")
            nc.sync.dma_start(lt[:], lt_d[:])
            iota16 = cpool.tile([128, 128], F16, tag="iota")
            nc.sync.dma_start(iota16[:], iota_d[:])
            dstage = cpool.tile([128, cfg.nch], F32, tag="dst", name="dstage")

            icol = 0
            bglob = 0
            qn = 0
            for s in range(cfg.nsup):
                chunks = cfg.sup_chunks[s]
                nchk = len(chunks)
                blocks = cfg.sup_blocks(s)
                nb = len(blocks)
                slab = slabp.tile([128, nb, ROW], F16, tag="slab")
                b0 = 0
                for g in range(cfg.ngrp):
                    gb = (cfg.caps[g] // 128) * nchk
                    done = 0
                    while done < gb:
                        pb = min(cfg.gpiece_idx // 128, gb - done)
                        n_idx = pb * 128
                        gcols = n_idx // 16
                        nc.gpsimd.dma_gather(
                            slab[:, b0 + done:b0 + done + pb, :],
                            v_d[g * cfg.group: g * cfg.group + cfg.grp_sizes[g], :],
                            idx_sb[:, icol:icol + gcols],
                            n_idx, n_idx, ROW, queue_num=qn % cfg.n_queues)
                        qn += 1
                        done += pb
                        icol += gcols
                    b0 += gb
                if _stage == "gather":
                    nc.sync.dma_start(dbg_d[:, bglob:bglob + nb, :], slab[:])
                    bglob += nb
                    continue
                ohb = op_.tile([128, nb, 128], F16, tag="ohb")
                nc.vector.tensor_tensor(
                    out=ohb[:],
                    in0=lt[:, bglob:bglob + nb].unsqueeze(-1).broadcast_to(
                        [128, nb, 128]),
                    in1=iota16[:].unsqueeze(1).broadcast_to([128, nb, 128]),
                    op=OP.is_equal)
                psts = [psp.tile([128, VC], F32, tag=f"s{i}", name=f"ps_{s}_{i}")
                        for i in range(nchk)]
                done_in_chunk = [0] * nchk
                per_chunk_total = cfg.bpc
                for b, (g, ci, j) in enumerate(blocks):
                    first = done_in_chunk[ci] == 0
                    last = done_in_chunk[ci] == per_chunk_total - 1
                    ps = psts[ci]
                    nc.tensor.matmul(out=ps[:], lhsT=ohb[:, b, :],
                                     rhs=slab[:, b, 0:VC],
                                     start=first, stop=last, skip_group_check=True)
                    done_in_chunk[ci] += 1
                    if last:
                        ch = chunks[ci]
                        d_sb = dstage[:, ch:ch + 1]
                        nc.vector.tensor_copy(d_sb, ps[:, c:c + 1])
                        dcol = evp.tile([128, 1], F32, tag="dcol")
                        nc.vector.scalar_tensor_tensor(
                            out=dcol[:], in0=d_sb, scalar=0.0,
                            in1=d_sb, op0=OP.is_equal, op1=OP.add)
                        rcol = evp.tile([128, 1], F32, tag="rcol")
                        nc.vector.reciprocal(rcol[:], dcol[:])
                        yt = yp.tile([128, c], F32, tag="yt")
                        nc.scalar.activation(yt[:], ps[:, 0:c], AF.Copy,
                                             scale=rcol[:])
                        nc.sync.dma_start(
                            y_d[ch * cfg.chunk:(ch + 1) * cfg.chunk, :], yt[:])
                bglob += nb
            if _stage == "full":
                nc.sync.dma_start(den_d[:], dstage[:])
    nc.compile()
    return nc


# ------------------------------------------------------------- host prep -----

def host_prep(cfg: Cfg, edges: np.ndarray):
    """Per-core dict of streams + overflow edge lists (same layout as v1)."""
    e = np.asarray(edges)
    tgt = e[:, 0].astype(np.int64)
    src = e[:, 1].astype(np.int64)
    core = tgt // cfg.tpc
    ltg = tgt % cfg.tpc
    chunk = ltg // cfg.chunk
    ltgt = ltg % cfg.chunk
    grp = src // cfg.group
    key = ((core * cfg.nch + chunk) * cfg.ngrp + grp)
    order = np.argsort(key, kind="stable")
    key_s = key[order]
    src_s = src[order]
    ltgt_s = ltgt[order]
    tgt_s = tgt[order]
    nruns = cfg.n_cores * cfg.nch * cfg.ngrp
    counts = np.bincount(key_s, minlength=nruns)
    starts = np.concatenate([[0], np.cumsum(counts)[:-1]])
    out = []
    for k in range(cfg.n_cores):
        idx_full = np.zeros(cfg.nblk * 128, np.int16)
        lt_full = np.full(cfg.nblk * 128, -1.0, np.float32)
        ovf = []
        for ch in range(cfg.nch):
            for g in range(cfg.ngrp):
                r = (k * cfg.nch + ch) * cfg.ngrp + g
                n = counts[r]
                s0 = starts[r]
                cap = cfg.caps[g]
                take = min(n, cap)
                sidx = ch // cfg.super
                ci = ch % cfg.super
                nchk = len(cfg.sup_chunks[sidx])
                blk0 = sum((cfg.caps[gg] // 128) * nchk for gg in range(g)) \
                    + ci * (cfg.caps[g] // 128)
                sup_blk0 = sum(len(cfg.sup_blocks(ss)) for ss in range(sidx))
                slot0 = (sup_blk0 + blk0) * 128
                idx_full[slot0:slot0 + take] = (src_s[s0:s0 + take] - g * cfg.group
                                                ).astype(np.int16)
                lt_full[slot0:slot0 + take] = ltgt_s[s0:s0 + take]
                if n > cap:
                    for t in range(s0 + cap, s0 + n):
                        ovf.append((int(tgt_s[t]), int(src_s[t])))
        # wrap idx stream per gather piece into [128, icols]
        idx_cols = []
        pos = 0
        for sidx in range(cfg.nsup):
            nchk = len(cfg.sup_chunks[sidx])
            for g in range(cfg.ngrp):
                gb = (cfg.caps[g] // 128) * nchk
                done = 0
                while done < gb:
                    pb = min(cfg.gpiece_idx // 128, gb - done)
                    n_idx = pb * 128
                    seg = idx_full[pos:pos + n_idx]
                    pos += n_idx
                    wrapped = seg.reshape(-1, 16).T   # [16, n/16]
                    idx_cols.append(np.tile(wrapped, (8, 1)))
                    done += pb
        idxs = np.concatenate(idx_cols, axis=1)
        assert idxs.shape == (128, cfg.icols), idxs.shape
        ltm = lt_full.reshape(cfg.nblk, 128).T.astype(np.float16)
        out.append(dict(IDX=idxs, LT=ltm, ovf=ovf))
    return out


def _install_ntff_shim():
    import types
    if "antenv.axon_hooks" not in sys.modules:
        mod = types.ModuleType("antenv.axon_hooks")
        state = {"hook": None}
        mod.set_axon_ntff_profile_hook = lambda h: state.__setitem__("hook", h)
        mod.get_axon_ntff_profile_hook = lambda: state["hook"]
        sys.modules["antenv.axon_hooks"] = mod
    mod = sys.modules["antenv.axon_hooks"]
    if mod.get_axon_ntff_profile_hook() is None:
        try:
            if "/root/.axon_site" not in sys.path:
                sys.path.insert(0, "/root/.axon_site")
            from trn_agent_boot.trn_boot import _ntff_profile_via_ctypes
            hook = _ntff_profile_via_ctypes("/opt/axon/libaxon_pjrt.so")
            if hook is not None:
                mod.set_axon_ntff_profile_hook(hook)
        except Exception as ex:
            print(f"NTFF shim failed: {ex}", file=sys.stderr)


_NC_CACHE = {}


def _get_ncs(cfg):
    key = (cfg.n_src, cfg.n_out, cfg.n_edges, cfg.n_cores)
    if key not in _NC_CACHE:
        _NC_CACHE[key] = (build_nc_A(cfg), build_nc_B(cfg))
    return _NC_CACHE[key]


def _run(nc, in_maps, cfg, trace=False):
    if trace:
        _install_ntff_shim()
    return run_bass_kernel_spmd(nc, in_maps, core_ids=list(range(cfg.n_cores)),
                                trace=trace)


def in_maps_A(cfg, x, W, b, att, alpha_f):
    tpc_src = cfg.n_src // cfg.n_cores
    tpc_pad = -(-tpc_src // 128) * 128
    x16 = np.asarray(x).astype(np.float16)
    wt16 = np.ascontiguousarray(W.T).astype(np.float16)
    attb = np.tile(att.astype(np.float32), (128, 1))
    brow = np.tile(b.astype(np.float32), (128, 1))
    c0 = float(att.astype(np.float64) @ b.astype(np.float64))
    cal = np.tile(np.array([c0, alpha_f], np.float32), (128, 1))
    maps = []
    for k in range(cfg.n_cores):
        xs = np.zeros((cfg.c, tpc_pad), np.float16)
        xs[:, 0:tpc_src] = x16[:, k * tpc_src:(k + 1) * tpc_src]
        maps.append(dict(x16=xs, WT16=wt16, ATTB=attb, BROW=brow, CAL=cal))
    return maps


def in_maps_B(cfg, V, prep):
    iota = np.tile(np.arange(128, dtype=np.float16), (128, 1))
    return [dict(V=V, IDX=prep[k]["IDX"], LT=prep[k]["LT"], IOTA16=iota)
            for k in range(cfg.n_cores)]


def assemble(cfg, results_B, prep, Vf):
    """Vf: [n_src, ROW] float16 view of V table (m rows | e)."""
    y = np.empty((cfg.c, cfg.n_out), np.float32)
    for k in range(cfg.n_cores):
        yk = results_B[k]["Y"]
        y[:, k * cfg.tpc:(k + 1) * cfg.tpc] = yk[0:cfg.tpc, :].T
    for k in range(cfg.n_cores):
        ovf = prep[k]["ovf"]
        if not ovf:
            continue
        den_k = results_B[k]["DEN"]
        acc = {}
        for (tg, sg) in ovf:
            mrow = Vf[sg, 0:cfg.c].astype(np.float32)
            ea = float(Vf[sg, cfg.c])
            if tg not in acc:
                acc[tg] = [0.0, np.zeros(cfg.c, np.float32)]
            acc[tg][0] += ea
            acc[tg][1] += mrow
        for tg, (sea, svec) in acc.items():
            ltg = tg % cfg.tpc
            ch, lp = ltg // cfg.chunk, ltg % cfg.chunk
            den_t = float(den_k[lp, ch])
            y[:, tg] = (y[:, tg] * den_t + svec) / (den_t + sea)
    return y


def kernel(x, edges, W, b, att, alpha, _trace=False, _cfg=None, _timing=None):
    cfg = _cfg or CFG
    x = np.asarray(x)
    W = np.asarray(W, dtype=np.float32)
    b = np.asarray(b, dtype=np.float32)
    att = np.asarray(att, dtype=np.float32)
    alpha_f = float(np.asarray(alpha))
    ncA, ncB = _get_ncs(cfg)

    # ---- launch A: build node table V ----
    resA = _run(ncA, in_maps_A(cfg, x, W, b, att, alpha_f), cfg, trace=_trace)
    tpc_src = cfg.n_src // cfg.n_cores
    Vf = np.concatenate(
        [np.asarray(resA.results[k]["V"])[0:tpc_src]
         for k in range(cfg.n_cores)], axis=0)
    V = Vf

    # ---- host prep of edge streams ----
    prep = host_prep(cfg, edges)

    # ---- launch B ----
    resB = _run(ncB, in_maps_B(cfg, V, prep), cfg, trace=_trace)

    if _timing is not None:
        _timing["A_ns"] = resA.exec_time_ns
        _timing["B_ns"] = resB.exec_time_ns

    return assemble(cfg, resB.results, prep, Vf)


# revision 3
# speedup vs baseline: 1.0785x; 1.0785x over previous
"""Trainium2 Bass kernel v2 for nn_BipartiteRemap (GNN attention message passing).

Key insight vs v1: the attention logit a_e = att.(W x_src + b) depends ONLY on
the source node, so exp(prelu(a)) = e_j and the per-edge message
m_e = e_j * (W x_j + b) are PER-NODE quantities.

  Launch A: per-node table V[j] = [m_j (128 f16) | e_j (f16) | pad] (512B rows).
  Launch B: per 128-edge block: dma_gather of V rows + one-hot(is_equal) +
    ONE accumulating matmul [128 slots x 129 cols] -> PSUM (num | den).
    Eviction: y = num / den.  Launch B is gather-descgen bound (Q7 ~6.5ns/idx);
    all other engines hide under it.
"""

import os
import sys

for _p in ("/opt/trn_rl_repo",):
    if _p not in sys.path:
        sys.path.insert(0, _p)

import numpy as np

import concourse.bass as bass
import concourse.bacc as bacc
import concourse.mybir as mybir
import concourse.tile as tile
from concourse import library_config
from concourse.bass_utils import run_bass_kernel_spmd

F32 = mybir.dt.float32
F16 = mybir.dt.float16
F8 = mybir.dt.float8e4
I16 = mybir.dt.int16
AF = mybir.ActivationFunctionType
OP = mybir.AluOpType

ROW = 256          # fp16 elems per V row (512 B)
VC = 129           # used cols per V row (m 128 + e 1)


class Cfg:
    def __init__(self, n_src=100_000, n_out=100_000, n_edges=1_600_000, c=128,
                 n_cores=8, chunk=128, super_chunks=5, group=32768,
                 cap_sigma=2.0, gpiece_idx=1024, n_queues=4, scratch=16384):
        self.n_queues = n_queues
        self.scratch = scratch
        self.n_src, self.n_out, self.n_edges, self.c = n_src, n_out, n_edges, c
        self.n_cores = n_cores
        self.chunk = chunk
        self.tpc = n_out // n_cores              # targets per core
        self.nch = -(-self.tpc // chunk)         # chunks per core
        self.super = super_chunks
        self.nsup = -(-self.nch // self.super)
        self.group = group
        self.ngrp = -(-n_src // group)
        self.grp_sizes = [min(group, n_src - g * group) for g in range(self.ngrp)]
        epc = n_edges / n_cores / self.nch       # mean edges per chunk
        self.caps = []
        for g in range(self.ngrp):
            mu = epc * self.grp_sizes[g] / n_src
            sig = mu ** 0.5
            cap = int(-(-(mu + cap_sigma * sig) // 128) * 128)
            self.caps.append(max(cap, 128))
        self.bpc = sum(self.caps) // 128         # blocks per chunk
        self.nblk = self.nch * self.bpc          # blocks per core
        self.icols = self.nblk * 128 // 16       # int16 idx columns
        self.gpiece_idx = gpiece_idx             # max idxs per dma_gather
        self.sup_chunks = [list(range(s * self.super, min((s + 1) * self.super, self.nch)))
                           for s in range(self.nsup)]

    def blk_pos(self, g, j):
        """position of block (g, j) within a chunk's g-major block list."""
        return sum(self.caps[gg] // 128 for gg in range(g)) + j

    def sup_blocks(self, s):
        """(g, ci_in_sup, j) in slab order for superchunk s (g-major)."""
        out = []
        for g in range(self.ngrp):
            for ci in range(len(self.sup_chunks[s])):
                for j in range(self.caps[g] // 128):
                    out.append((g, ci, j))
        return out


CFG = Cfg()


# ---------------------------------------------------------------- launch A ---

def build_nc_A(cfg: Cfg, sup=7):
    nc = bacc.Bacc("TRN2", target_bir_lowering=False, debug=False,
                   enable_asserts=False, num_devices=cfg.n_cores)
    c = cfg.c
    tpc_src = -(-cfg.n_src // cfg.n_cores // 128) * 128   # host pads to 12544
    nb = tpc_src // 128
    assert nb % sup == 0
    x_d = nc.dram_tensor("x16", [c, tpc_src], F16, kind="ExternalInput")
    wt_d = nc.dram_tensor("WT16", [c, c], F16, kind="ExternalInput")
    attb_d = nc.dram_tensor("ATTB", [128, c], F32, kind="ExternalInput")
    brow_d = nc.dram_tensor("BROW", [128, c], F32, kind="ExternalInput")
    cal_d = nc.dram_tensor("CAL", [128, 2], F32, kind="ExternalInput")  # c0 | alpha
    v_d = nc.dram_tensor("V", [tpc_src, ROW], F16, kind="ExternalOutput")
    with tile.TileContext(nc) as tc:
        import contextlib
        with contextlib.ExitStack() as ctx:
            cpool = ctx.enter_context(tc.tile_pool(name="c", bufs=1))
            ppool = ctx.enter_context(tc.tile_pool(name="p", bufs=2, space="PSUM"))
            spool = ctx.enter_context(tc.tile_pool(name="s", bufs=3))
            vpool = ctx.enter_context(tc.tile_pool(name="v", bufs=3))
            wt = cpool.tile([c, c], F16, tag="wt")
            nc.sync.dma_start(wt[:], wt_d[:])
            attb = cpool.tile([128, c], F32, tag="attb")
            nc.sync.dma_start(attb[:], attb_d[:])
            brow = cpool.tile([128, c], F32, tag="brow")
            nc.sync.dma_start(brow[:], brow_d[:])
            cal = cpool.tile([128, 2], F32, tag="cal")
            nc.sync.dma_start(cal[:], cal_d[:])
            c0col = cal[:, 0:1]
            alcol = cal[:, 1:2]
            x_sb = cpool.tile([c, tpc_src], F16, tag="x")
            nc.sync.dma_start(x_sb[:], x_d[:])
            for s in range(nb // sup):
                ps = ppool.tile([128, sup, c], F32, tag="ps")
                for i in range(sup):
                    o = (s * sup + i) * 128
                    nc.tensor.matmul(out=ps[:, i, :],
                                     lhsT=x_sb[:, o:o + 128], rhs=wt[:],
                                     start=True, stop=True)
                prod = spool.tile([128, sup, c], F32, tag="prod")
                nc.vector.tensor_tensor(
                    out=prod[:], in0=ps[:],
                    in1=attb[:].unsqueeze(1).broadcast_to([128, sup, c]),
                    op=OP.mult)
                acol = spool.tile([128, sup], F32, tag="acol")
                nc.vector.tensor_reduce(out=acol[:], in_=prod[:],
                                        axis=mybir.AxisListType.X, op=OP.add)
                lcol = spool.tile([128, sup], F32, tag="lcol")
                nc.vector.tensor_scalar(
                    out=lcol[:], in0=acol[:], scalar1=c0col,
                    scalar2=None, op0=OP.add)
                pcol = spool.tile([128, sup], F32, tag="pcol")
                nc.vector.scalar_tensor_tensor(
                    out=pcol[:], in0=lcol[:], scalar=alcol,
                    in1=lcol[:], op0=OP.mult, op1=OP.max)
                ecol = spool.tile([128, sup], F32, tag="ecol")
                nc.scalar.activation(ecol[:], pcol[:], AF.Exp)
                fix = spool.tile([128, sup, c], F32, tag="fix")
                nc.vector.tensor_tensor(
                    out=fix[:], in0=ps[:],
                    in1=brow[:].unsqueeze(1).broadcast_to([128, sup, c]),
                    op=OP.add)
                vt = vpool.tile([128, sup, ROW], F16, tag="vt")
                nc.vector.tensor_tensor(
                    out=vt[:, :, 0:c], in0=fix[:],
                    in1=ecol[:].unsqueeze(-1).broadcast_to([128, sup, c]),
                    op=OP.mult)
                nc.vector.tensor_copy(vt[:, :, c:c + 1], ecol[:].unsqueeze(-1))
                nc.sync.dma_start(
                    v_d[s * sup * 128:(s + 1) * sup * 128, :].rearrange(
                        "(b p) r -> p b r", p=128),
                    vt[:])
    nc.compile()
    return nc


# ---------------------------------------------------------------- launch B ---

def build_nc_B(cfg: Cfg, _stage="full"):
    nc = bacc.Bacc("TRN2", target_bir_lowering=False, debug=False,
                   enable_asserts=False, num_devices=cfg.n_cores,
                   num_swdge_queues=cfg.n_queues,
                   dynamic_dma_scratch_size=cfg.scratch)
    c = cfg.c
    if _stage != "full":
        dbg_d = nc.dram_tensor("DBG", [128, cfg.nblk, ROW], I16, kind="ExternalOutput")
    v_d = nc.dram_tensor("V", [cfg.n_src, ROW], F16, kind="ExternalInput")
    idx_d = nc.dram_tensor("IDX", [128, cfg.icols], I16, kind="ExternalInput")
    lt_d = nc.dram_tensor("LT", [128, cfg.nblk], F32, kind="ExternalInput")
    iota_d = nc.dram_tensor("IOTA16", [128, 128], F16, kind="ExternalInput")
    y_d = nc.dram_tensor("Y", [cfg.nch * cfg.chunk, c], F32, kind="ExternalOutput")
    den_d = nc.dram_tensor("DEN", [128, cfg.nch], F32, kind="ExternalOutput")

    with tile.TileContext(nc) as tc:
        import contextlib
        with contextlib.ExitStack() as ctx:
            cpool = ctx.enter_context(tc.tile_pool(name="const", bufs=1))
            slabp = ctx.enter_context(tc.tile_pool(name="slab", bufs=2))
            op_ = ctx.enter_context(tc.tile_pool(name="oh", bufs=6))
            psp = ctx.enter_context(tc.tile_pool(name="ps", bufs=1, space="PSUM"))
            evp = ctx.enter_context(tc.tile_pool(name="ev", bufs=4))
            yp = ctx.enter_context(tc.tile_pool(name="y", bufs=3))

            nc.gpsimd.load_library(library_config.mlp)

            idx_sb = cpool.tile([128, cfg.icols], I16, tag="idx")
            nc.sync.dma_start(idx_sb[:], idx_d[:])
            lt = cpool.tile([128, cfg.nblk], F32, tag="lt")
            nc.sync.dma_start(lt[:], lt_d[:])
            iota16 = cpool.tile([128, 128], F16, tag="iota")
            nc.sync.dma_start(iota16[:], iota_d[:])
            dstage = cpool.tile([128, cfg.nch], F32, tag="dst", name="dstage")

            icol = 0
            bglob = 0
            qn = 0
            for s in range(cfg.nsup):
                chunks = cfg.sup_chunks[s]
                nchk = len(chunks)
                blocks = cfg.sup_blocks(s)
                nb = len(blocks)
                slab = slabp.tile([128, nb, ROW], F16, tag="slab")
                b0 = 0
                for g in range(cfg.ngrp):
                    gb = (cfg.caps[g] // 128) * nchk
                    done = 0
                    while done < gb:
                        pb = min(cfg.gpiece_idx // 128, gb - done)
                        n_idx = pb * 128
                        gcols = n_idx // 16
                        nc.gpsimd.dma_gather(
                            slab[:, b0 + done:b0 + done + pb, :],
                            v_d[g * cfg.group: g * cfg.group + cfg.grp_sizes[g], :],
                            idx_sb[:, icol:icol + gcols],
                            n_idx, n_idx, ROW, queue_num=qn % cfg.n_queues)
                        qn += 1
                        done += pb
                        icol += gcols
                    b0 += gb
                if _stage == "gather":
                    nc.sync.dma_start(dbg_d[:, bglob:bglob + nb, :], slab[:])
                    bglob += nb
                    continue
                psts = [psp.tile([128, VC], F32, tag=f"s{i}", name=f"ps_{s}_{i}")
                        for i in range(nchk)]
                done_in_chunk = [0] * nchk
                per_chunk_total = cfg.bpc
                for b, (g, ci, j) in enumerate(blocks):
                    gb_i = bglob + b
                    oh = op_.tile([128, 128], F16, tag="oh")
                    nc.vector.tensor_scalar(
                        out=oh[:], in0=iota16[:], scalar1=lt[:, gb_i:gb_i + 1],
                        scalar2=None, op0=OP.is_equal)
                    first = done_in_chunk[ci] == 0
                    last = done_in_chunk[ci] == per_chunk_total - 1
                    ps = psts[ci]
                    nc.tensor.matmul(out=ps[:], lhsT=oh[:],
                                     rhs=slab[:, b, 0:VC],
                                     start=first, stop=last, skip_group_check=True)
                    done_in_chunk[ci] += 1
                    if last:
                        ch = chunks[ci]
                        d_sb = dstage[:, ch:ch + 1]
                        nc.vector.tensor_copy(d_sb, ps[:, c:c + 1])
                        dcol = evp.tile([128, 1], F32, tag="dcol")
                        nc.vector.scalar_tensor_tensor(
                            out=dcol[:], in0=d_sb, scalar=0.0,
                            in1=d_sb, op0=OP.is_equal, op1=OP.add)
                        rcol = evp.tile([128, 1], F32, tag="rcol")
                        nc.vector.reciprocal(rcol[:], dcol[:])
                        yt = yp.tile([128, c], F32, tag="yt")
                        nc.scalar.activation(yt[:], ps[:, 0:c], AF.Copy,
                                             scale=rcol[:])
                        nc.sync.dma_start(
                            y_d[ch * cfg.chunk:(ch + 1) * cfg.chunk, :], yt[:])
                bglob += nb
            if _stage == "full":
                nc.sync.dma_start(den_d[:], dstage[:])
    nc.compile()
    return nc


# ------------------------------------------------------------- host prep -----

def host_prep(cfg: Cfg, edges: np.ndarray):
    """Per-core dict of streams + overflow edge lists (same layout as v1)."""
    e = np.asarray(edges)
    tgt = e[:, 0].astype(np.int64)
    src = e[:, 1].astype(np.int64)
    core = tgt // cfg.tpc
    ltg = tgt % cfg.tpc
    chunk = ltg // cfg.chunk
    ltgt = ltg % cfg.chunk
    grp = src // cfg.group
    key = ((core * cfg.nch + chunk) * cfg.ngrp + grp)
    order = np.argsort(key, kind="stable")
    key_s = key[order]
    src_s = src[order]
    ltgt_s = ltgt[order]
    tgt_s = tgt[order]
    nruns = cfg.n_cores * cfg.nch * cfg.ngrp
    counts = np.bincount(key_s, minlength=nruns)
    starts = np.concatenate([[0], np.cumsum(counts)[:-1]])
    out = []
    for k in range(cfg.n_cores):
        idx_full = np.zeros(cfg.nblk * 128, np.int16)
        lt_full = np.full(cfg.nblk * 128, -1.0, np.float32)
        ovf = []
        for ch in range(cfg.nch):
            for g in range(cfg.ngrp):
                r = (k * cfg.nch + ch) * cfg.ngrp + g
                n = counts[r]
                s0 = starts[r]
                cap = cfg.caps[g]
                take = min(n, cap)
                sidx = ch // cfg.super
                ci = ch % cfg.super
                nchk = len(cfg.sup_chunks[sidx])
                blk0 = sum((cfg.caps[gg] // 128) * nchk for gg in range(g)) \
                    + ci * (cfg.caps[g] // 128)
                sup_blk0 = sum(len(cfg.sup_blocks(ss)) for ss in range(sidx))
                slot0 = (sup_blk0 + blk0) * 128
                idx_full[slot0:slot0 + take] = (src_s[s0:s0 + take] - g * cfg.group
                                                ).astype(np.int16)
                lt_full[slot0:slot0 + take] = ltgt_s[s0:s0 + take]
                if n > cap:
                    for t in range(s0 + cap, s0 + n):
                        ovf.append((int(tgt_s[t]), int(src_s[t])))
        # wrap idx stream per gather piece into [128, icols]
        idx_cols = []
        pos = 0
        for sidx in range(cfg.nsup):
            nchk = len(cfg.sup_chunks[sidx])
            for g in range(cfg.ngrp):
                gb = (cfg.caps[g] // 128) * nchk
                done = 0
                while done < gb:
                    pb = min(cfg.gpiece_idx // 128, gb - done)
                    n_idx = pb * 128
                    seg = idx_full[pos:pos + n_idx]
                    pos += n_idx
                    wrapped = seg.reshape(-1, 16).T   # [16, n/16]
                    idx_cols.append(np.tile(wrapped, (8, 1)))
                    done += pb
        idxs = np.concatenate(idx_cols, axis=1)
        assert idxs.shape == (128, cfg.icols), idxs.shape
        ltm = lt_full.reshape(cfg.nblk, 128).T.copy()
        out.append(dict(IDX=idxs, LT=ltm, ovf=ovf))
    return out


def _install_ntff_shim():
    import types
    if "antenv.axon_hooks" not in sys.modules:
        mod = types.ModuleType("antenv.axon_hooks")
        state = {"hook": None}
        mod.set_axon_ntff_profile_hook = lambda h: state.__setitem__("hook", h)
        mod.get_axon_ntff_profile_hook = lambda: state["hook"]
        sys.modules["antenv.axon_hooks"] = mod
    mod = sys.modules["antenv.axon_hooks"]
    if mod.get_axon_ntff_profile_hook() is None:
        try:
            if "/root/.axon_site" not in sys.path:
                sys.path.insert(0, "/root/.axon_site")
            from trn_agent_boot.trn_boot import _ntff_profile_via_ctypes
            hook = _ntff_profile_via_ctypes("/opt/axon/libaxon_pjrt.so")
            if hook is not None:
                mod.set_axon_ntff_profile_hook(hook)
        except Exception as ex:
            print(f"NTFF shim failed: {ex}", file=sys.stderr)


_NC_CACHE = {}


def _get_ncs(cfg):
    key = (cfg.n_src, cfg.n_out, cfg.n_edges, cfg.n_cores)
    if key not in _NC_CACHE:
        _NC_CACHE[key] = (build_nc_A(cfg), build_nc_B(cfg))
    return _NC_CACHE[key]


def _run(nc, in_maps, cfg, trace=False):
    if trace:
        _install_ntff_shim()
    return run_bass_kernel_spmd(nc, in_maps, core_ids=list(range(cfg.n_cores)),
                                trace=trace)


def in_maps_A(cfg, x, W, b, att, alpha_f):
    tpc_src = cfg.n_src // cfg.n_cores
    tpc_pad = -(-tpc_src // 128) * 128
    x16 = np.asarray(x).astype(np.float16)
    wt16 = np.ascontiguousarray(W.T).astype(np.float16)
    attb = np.tile(att.astype(np.float32), (128, 1))
    brow = np.tile(b.astype(np.float32), (128, 1))
    c0 = float(att.astype(np.float64) @ b.astype(np.float64))
    cal = np.tile(np.array([c0, alpha_f], np.float32), (128, 1))
    maps = []
    for k in range(cfg.n_cores):
        xs = np.zeros((cfg.c, tpc_pad), np.float16)
        xs[:, 0:tpc_src] = x16[:, k * tpc_src:(k + 1) * tpc_src]
        maps.append(dict(x16=xs, WT16=wt16, ATTB=attb, BROW=brow, CAL=cal))
    return maps


def in_maps_B(cfg, V, prep):
    iota = np.tile(np.arange(128, dtype=np.float16), (128, 1))
    return [dict(V=V, IDX=prep[k]["IDX"], LT=prep[k]["LT"], IOTA16=iota)
            for k in range(cfg.n_cores)]


def assemble(cfg, results_B, prep, Vf):
    """Vf: [n_src, ROW] float16 view of V table (m rows | e)."""
    y = np.empty((cfg.c, cfg.n_out), np.float32)
    for k in range(cfg.n_cores):
        yk = results_B[k]["Y"]
        y[:, k * cfg.tpc:(k + 1) * cfg.tpc] = yk[0:cfg.tpc, :].T
    for k in range(cfg.n_cores):
        ovf = prep[k]["ovf"]
        if not ovf:
            continue
        den_k = results_B[k]["DEN"]
        acc = {}
        for (tg, sg) in ovf:
            mrow = Vf[sg, 0:cfg.c].astype(np.float32)
            ea = float(Vf[sg, cfg.c])
            if tg not in acc:
                acc[tg] = [0.0, np.zeros(cfg.c, np.float32)]
            acc[tg][0] += ea
            acc[tg][1] += mrow
        for tg, (sea, svec) in acc.items():
            ltg = tg % cfg.tpc
            ch, lp = ltg // cfg.chunk, ltg % cfg.chunk
            den_t = float(den_k[lp, ch])
            y[:, tg] = (y[:, tg] * den_t + svec) / (den_t + sea)
    return y


def kernel(x, edges, W, b, att, alpha, _trace=False, _cfg=None, _timing=None):
    cfg = _cfg or CFG
    x = np.asarray(x)
    W = np.asarray(W, dtype=np.float32)
    b = np.asarray(b, dtype=np.float32)
    att = np.asarray(att, dtype=np.float32)
    alpha_f = float(np.asarray(alpha))
    ncA, ncB = _get_ncs(cfg)

    # ---- launch A: build node table V ----
    resA = _run(ncA, in_maps_A(cfg, x, W, b, att, alpha_f), cfg, trace=_trace)
    tpc_src = cfg.n_src // cfg.n_cores
    Vf = np.concatenate(
        [np.asarray(resA.results[k]["V"])[0:tpc_src]
         for k in range(cfg.n_cores)], axis=0)
    V = Vf

    # ---- host prep of edge streams ----
    prep = host_prep(cfg, edges)

    # ---- launch B ----
    resB = _run(ncB, in_maps_B(cfg, V, prep), cfg, trace=_trace)

    if _timing is not None:
        _timing["A_ns"] = resA.exec_time_ns
        _timing["B_ns"] = resB.exec_time_ns

    return assemble(cfg, resB.results, prep, Vf)
